# revision 1
# baseline (speedup 1.0000x reference)
"""Trainium2 Bass kernel for nn_Memory_22548578304755 (scatter_memory).

Computes: mean_b [ -log_softmax(mask(inputs @ features.T / temp))[b, indices[b]] ]

Strategy (8 NeuronCores, SPMD):
  - Shard the feature bank row-wise: core c owns rows [c*12500, (c+1)*12500),
    zero-padded to 13000 (padding columns produce exp(-C_SHIFT) = 0).
  - Host pre-transposes each shard to [D, 13000] and quantizes to
    fp8e3m4 scaled by 64 so matmul operands have the contraction dim (D)
    on SBUF partitions; the inputs operand stays fp16 (mixed-dtype matmul).
  - The intra-camera mask is folded into the matmul: 8 extra one-hot
    "camera" rows are appended to the contraction. The inputs side carries
    BIG*64 * onehot(camids_batch), the features side carries onehot(camids).
    Matching camids add +BIG to the (descaled) score; after the fixed shift
    exp(score - (BIG + K)) non-matching entries underflow to 0 exactly.
  - B=64 uses only half the PE array columns, so each chunk computes TWO
    500-column score groups concurrently via PE column tiling:
    tile_position (0,0) -> PSUM partitions 0..63, (0,64) -> 64..127.
  - Each core computes s_c[b] = sum_n exp(aug_score[b,n] - C_SHIFT) via
    PSUM -> ScalarE exp-with-accumulate (scale=1/64 descales); the host
    combines the 8 partial denominators (cross-device logsumexp) with the
    on-device fp32 target-score dot into the final scalar.
"""

import sys

import numpy as np

sys.path.insert(0, "/opt/trn_rl_repo")

import ml_dtypes  # noqa: E402

import concourse.bacc as bacc  # noqa: E402
import concourse.mybir as mybir  # noqa: E402
from concourse.tile import TileContext  # noqa: E402
from concourse.bass_utils import run_bass_kernel_spmd  # noqa: E402

B = 64
N = 100000
D = 2048
NCAMS = 8
TEMP = 0.07
NCORES = 8
N_SHARD_RAW = N // NCORES  # 12500
N_SHARD = 13000  # zero-padded so 500-col groups pair up for col-tiling

BIG = 512.0  # mask offset added to same-camera scores
K_SHIFT = 100.0  # extra shift so exp never overflows
C_SHIFT = BIG + K_SHIFT
FEAT_SCALE = 64.0  # fp8 feature pre-scale (power of 2; BIG*FEAT_SCALE fp16-exact)

KC = D // 128  # 16 full contraction chunks
N_MM = 500  # matmul moving free-dim (one PSUM bank)
CHUNKS = (1000,) * 13  # DMA chunk schedule; each chunk = one col-tiled pair


def build_nc(n_shard: int, chunks=CHUNKS, n_mm: int = N_MM):
    """Build the single-core Bass program (identical across the 8 cores)."""
    assert sum(chunks) == n_shard and all(c % (2 * n_mm) == 0 for c in chunks)
    max_chunk = max(chunks)
    n_pairs = n_shard // (2 * n_mm)

    dt = mybir.dt
    nc = bacc.Bacc()

    featT = nc.declare_dram_parameter("featT", [D, n_shard], dt.float8e3, False)
    featC = nc.declare_dram_parameter("featC", [NCAMS, n_shard], dt.float16, False)
    inpP = nc.declare_dram_parameter("inpP", [128, (KC + 1) * B], dt.float16, False)
    gath = nc.declare_dram_parameter("gath", [B, D + NCAMS], dt.float32, False)
    xnat = nc.declare_dram_parameter("xnat", [B, D + NCAMS], dt.float32, False)
    out = nc.declare_dram_parameter("out", [128, n_pairs + 1], dt.float32, True)
    

    with TileContext(nc) as tc:
        with (
            tc.tile_pool(name="feat", bufs=4) as featp,
            tc.tile_pool(name="small", bufs=1) as smallp,
            tc.tile_pool(name="scratch", bufs=4) as scrp,
            tc.tile_pool(name="psum", bufs=6, space="PSUM") as psump,
        ):
            inp_t = smallp.tile([128, (KC + 1) * B], dt.float16)
            nc.scalar.dma_start(inp_t[:], inpP[:, :])
            partials = smallp.tile([128, n_pairs + 1], dt.float32)
            nbias = smallp.tile([128, 1], dt.float32)
            nc.vector.memset(nbias[:], -C_SHIFT)
            nc.vector.memset(partials[:], 0.0)

            # Target-score row dot: tsel[b] = sum(gath[b] * xnat[b]).
            g_t = smallp.tile([B, D + NCAMS], dt.float32)
            x_t = smallp.tile([B, D + NCAMS], dt.float32)
            nc.scalar.dma_start(g_t[:], gath[:, :])
            nc.scalar.dma_start(x_t[:], xnat[:, :])
            nc.vector.tensor_mul(g_t[:], g_t[:], x_t[:])
            nc.vector.reduce_sum(
                out=partials[0:B, n_pairs : n_pairs + 1],
                in_=g_t[:],
                axis=mybir.AxisListType.X,
            )

            c0 = 0
            pi = 0
            for ci, csz in enumerate(chunks):
                dma_eng = nc.sync if ci % 2 == 0 else nc.gpsimd
                ft = featp.tile([128, KC, max_chunk], dt.float8e3, tag="ft")
                src = featT[:, c0 : c0 + csz].rearrange("(kc p) n -> p kc n", p=128)
                dma_eng.dma_start(ft[:, :, :csz], src)
                camc = scrp.tile([NCAMS, max_chunk], dt.float16, tag="camc")
                dma_eng.dma_start(camc[:, :csz], featC[:, c0 : c0 + csz])

                lhs8 = inp_t[0:NCAMS, KC * B : KC * B + B]
                for si in range(csz // (2 * n_mm)):
                    oa = 2 * si * n_mm
                    ob = oa + n_mm
                    ps = psump.tile([128, n_mm], dt.float32, tag="ps")
                    for k in range(KC):
                        lhs = inp_t[:, k * B : (k + 1) * B]
                        nc.tensor.matmul(
                            ps[0:B, :],
                            lhs,
                            ft[:, k, oa : oa + n_mm],
                            start=(k == 0),
                            stop=False,
                            tile_position=(0, 0),
                        )
                        nc.tensor.matmul(
                            ps[B : 2 * B, :],
                            lhs,
                            ft[:, k, ob : ob + n_mm],
                            start=(k == 0),
                            stop=False,
                            tile_position=(0, B),
                        )
                    nc.tensor.matmul(
                        ps[0:B, :],
                        lhs8,
                        camc[:, oa : oa + n_mm],
                        start=False,
                        stop=True,
                        tile_position=(0, 0),
                    )
                    nc.tensor.matmul(
                        ps[B : 2 * B, :],
                        lhs8,
                        camc[:, ob : ob + n_mm],
                        start=False,
                        stop=True,
                        tile_position=(0, B),
                    )
                    ex = scrp.tile([128, n_mm], dt.float32, tag="ex")
                    nc.scalar.activation(
                        ex[:],
                        ps[:],
                        mybir.ActivationFunctionType.Exp,
                        bias=nbias[:],
                        scale=1.0 / FEAT_SCALE,
                        accum_out=partials[:, pi : pi + 1],
                    )
                    pi += 1
                c0 += csz

            nc.sync.dma_start(out[:, :], partials[:])
    nc.finalize()
    return nc


def _prep_host(inputs, features, indices, camids, camids_batch, n_shard_raw, n_shard):
    """Host-side shard prep. Returns per-core in_maps."""
    f16 = np.float16
    f8 = ml_dtypes.float8_e3m4
    x = np.asarray(inputs, np.float32) / TEMP  # [B, D]
    cb = np.asarray(camids_batch).astype(np.int64)
    cn = np.asarray(camids).astype(np.int64)
    idx = np.asarray(indices).astype(np.int64)

    oh_b = (cb[:, None] == np.arange(NCAMS)[None, :]).astype(np.float32)  # [B, 8]

    # Packed lhsT: [128, 17*64]; block k<16 = x.T rows, block 16 = aug rows.
    inpP = np.zeros((128, (KC + 1) * B), np.float32)
    xt = x.T  # [D, B]
    for k in range(KC):
        inpP[:, k * B : (k + 1) * B] = xt[k * 128 : (k + 1) * 128, :]
    inpP[:NCAMS, KC * B : KC * B + B] = (BIG * FEAT_SCALE * oh_b).T
    inpP = np.ascontiguousarray(inpP.astype(f16))

    gathered = np.concatenate(
        [np.asarray(features, np.float32)[idx], oh_b], axis=1
    ).astype(np.float32)  # [B, D+8]
    xnat = np.concatenate([x, BIG * oh_b], axis=1).astype(np.float32)

    ncores = features.shape[0] // n_shard_raw
    pad = n_shard - n_shard_raw
    in_maps = []
    for c in range(ncores):
        sl = slice(c * n_shard_raw, (c + 1) * n_shard_raw)
        fT = np.asarray(features[sl], np.float32).T * FEAT_SCALE  # [D, raw]
        if pad:
            fT = np.concatenate([fT, np.zeros((D, pad), np.float32)], axis=1)
        fT = np.ascontiguousarray(fT.astype(f8))
        fC = (cn[sl][None, :] == np.arange(NCAMS)[:, None]).astype(np.float32)
        if pad:
            fC = np.concatenate([fC, np.zeros((NCAMS, pad), np.float32)], axis=1)
        fC = np.ascontiguousarray(fC.astype(f16))
        in_maps.append(
            {"featT": fT, "featC": fC, "inpP": inpP, "gath": gathered, "xnat": xnat}
        )
    return in_maps


def _combine_host(results):
    """Cross-core logsumexp combine -> final scalar."""
    raw = np.stack([r["out"] for r in results]).astype(np.float64)  # [nc, 128, P+1]
    per_core = raw[:, :, :-1].sum(axis=2)  # [ncores, 128]
    s = per_core[:, :B] + per_core[:, B:]  # add the two col-tile halves
    tsel = raw[0, :B, -1]  # [B] = score_target + BIG
    stot = s.sum(axis=0)
    lse = np.log(stot) + C_SHIFT  # = logsumexp of aug scores
    nll = lse - tsel
    return np.float32(nll.mean())


_NC_CACHE = {}


def _get_nc(n_shard, chunks, n_mm):
    key = (n_shard, tuple(chunks), n_mm)
    if key not in _NC_CACHE:
        _NC_CACHE[key] = build_nc(n_shard, chunks, n_mm)
    return _NC_CACHE[key]


def run_device(in_maps, n_shard, chunks=CHUNKS, n_mm=N_MM, **kwargs):
    nc = _get_nc(n_shard, chunks, n_mm)
    return run_bass_kernel_spmd(
        nc, in_maps, core_ids=list(range(len(in_maps))), **kwargs
    )


def kernel(inputs, features, indices, camids, camids_batch):
    in_maps = _prep_host(
        inputs, features, indices, camids, camids_batch, N_SHARD_RAW, N_SHARD
    )
    res = run_device(in_maps, N_SHARD)
    return _combine_host(res.results)



# revision 6
# speedup vs baseline: 1.0446x; 1.0446x over previous
"""Trainium2 Bass kernel for nn_Memory_22548578304755 (scatter_memory).

Computes: mean_b [ -log_softmax(mask(inputs @ features.T / temp))[b, indices[b]] ]

Strategy (8 NeuronCores, SPMD):
  - Shard the feature bank row-wise: core c owns rows [c*12500, (c+1)*12500).
    12500 = 25 groups of 500 score columns -> 12 col-tiled pairs (1000 cols)
    plus one single 500-col group; no padding.
  - Host pre-transposes each shard, quantizes to fp8e4m3 scaled by 64, and
    lays it out chunk-major ([12, 128, KC, 1000] + [128, KC, 500]) so every
    chunk DMA is a fully contiguous 16KB-per-partition read. Chunk DMAs are
    spread round-robin over 3 queues (sync / gpsimd / vector).
  - The query block is quantized to fp8e4m3 too, enabling DoubleRow matmuls:
    each instruction contracts 2x128 rows at 0.5 cyc/col, cutting PE busy
    ~4x vs the fp16 x fp8e3 path so the kernel is purely DMA-bound.
  - The intra-camera mask is folded into the matmul: 8 extra one-hot
    "camera" rows are appended to the contraction. The inputs side carries
    BIG*64 * onehot(camids_batch) (fp16), the features side onehot(camids)
    (fp8e4). Matching camids add +BIG to the (descaled) score; after the
    fixed shift exp(score - (BIG + K)) non-matching entries underflow to 0.
  - Each core computes s_c[b] = sum_n exp(aug_score[b,n] - C_SHIFT) via
    PSUM -> ScalarE exp-with-accumulate (scale=1/64 descales); the host
    combines the 8 partial denominators (cross-device logsumexp) and
    subtracts the exact fp64 target scores for the final scalar.
"""

import sys

import numpy as np

sys.path.insert(0, "/opt/trn_rl_repo")

import ml_dtypes  # noqa: E402

import concourse.bacc as bacc  # noqa: E402
import concourse.mybir as mybir  # noqa: E402
from concourse.tile import TileContext  # noqa: E402
from concourse.bass_utils import run_bass_kernel_spmd  # noqa: E402

B = 64
N = 100000
D = 2048
NCAMS = 8
TEMP = 0.07
NCORES = 8
N_SHARD = N // NCORES  # 12500

BIG = 512.0  # mask offset added to same-camera scores
K_SHIFT = 100.0  # extra shift so exp never overflows
C_SHIFT = BIG + K_SHIFT
FEAT_SCALE = 64.0  # fp8 feature pre-scale (power of 2)

KC = D // 128  # 16 contraction chunks of 128 (8 DoubleRow pairs)
N_MM = 500  # matmul moving free-dim (one PSUM bank)
NPAIR = 12  # 1000-col DMA chunks; final 500-col chunk is single
NGROUPS = 25  # 500-col score groups (12*2 + 1)

# queue per chunk DMA (13 entries). scalar also runs the exp activations,
# which sit behind its chunk DMAs in queue order: a scalar chunk at index c
# is deadlock-free iff c < feat_bufs + psum_bufs (its buffer-recycle wait
# never depends on an activation). feat_bufs=6, psum_bufs=6 -> c < 12.
PLAN = ("sync", "gpsimd") * 7


def build_nc(plan=PLAN):
    """Build the single-core Bass program (identical across the 8 cores)."""
    dt = mybir.dt
    DR = mybir.MatmulPerfMode.DoubleRow
    nc = bacc.Bacc()

    featA = nc.declare_dram_parameter("featA", [NPAIR, 128, KC, 1000], dt.float8e4, False)
    featB = nc.declare_dram_parameter("featB", [128, KC, N_MM], dt.float8e4, False)
    featC = nc.declare_dram_parameter("featC", [NCAMS, N_SHARD], dt.float8e4, False)
    inp8 = nc.declare_dram_parameter("inp8", [128, KC, B], dt.float8e4, False)
    aug16 = nc.declare_dram_parameter("aug16", [NCAMS, B], dt.float16, False)
    out = nc.declare_dram_parameter("out", [B, NGROUPS], dt.float32, True)

    with TileContext(nc) as tc:
        with (
            tc.tile_pool(name="feat", bufs=6) as featp,
            tc.tile_pool(name="small", bufs=1) as smallp,
            tc.tile_pool(name="scratch", bufs=3) as scrp,
            tc.tile_pool(name="psum", bufs=6, space="PSUM") as psump,
        ):
            inp_t = smallp.tile([128, KC, B], dt.float8e4)
            aug_t = smallp.tile([NCAMS, B], dt.float16)
            camc = smallp.tile([NCAMS, N_SHARD], dt.float8e4)
            nc.scalar.dma_start(inp_t[:], inp8[:, :, :])
            nc.scalar.dma_start(aug_t[:], aug16[:, :])
            nc.scalar.dma_start(camc[:], featC[:, :])
            partials = smallp.tile([B, NGROUPS], dt.float32)
            nbias = smallp.tile([B, 1], dt.float32)
            nc.vector.memset(nbias[:], -C_SHIFT)
            nc.vector.memset(partials[:], 0.0)

            queues = {"sync": nc.sync, "gpsimd": nc.gpsimd, "scalar": nc.scalar}
            lastt = smallp.tile([128, KC, N_MM], dt.float8e4)
            fts = []
            for c in range(NPAIR):
                ft = featp.tile([128, KC, 1000], dt.float8e4, tag="ft")
                queues[plan[c]].dma_start(ft[:], featA[c, :, :, :])
                fts.append(ft)
            queues[plan[NPAIR]].dma_start(lastt[:], featB[:, :, :])

            for g in range(NGROUPS):
                c, half = divmod(g, 2)
                if c < NPAIR:
                    ft = fts[c]
                    cols = slice(half * N_MM, (half + 1) * N_MM)
                else:
                    ft = lastt
                    cols = slice(0, N_MM)
                ps = psump.tile([B, N_MM], dt.float32, tag="ps")
                for kk in range(KC // 2):
                    nc.tensor.matmul(
                        ps[:, :],
                        inp_t[:, 2 * kk : 2 * kk + 2, :],
                        ft[:, 2 * kk : 2 * kk + 2, cols],
                        start=(kk == 0),
                        stop=False,
                        perf_mode=DR,
                    )
                nc.tensor.matmul(
                    ps[:, :],
                    aug_t[:],
                    camc[:, g * N_MM : (g + 1) * N_MM],
                    start=False,
                    stop=True,
                )
                ex = scrp.tile([B, N_MM], dt.float32, tag="ex")
                nc.scalar.activation(
                    ex[:, :],
                    ps[:, :],
                    mybir.ActivationFunctionType.Exp,
                    bias=nbias[0:B, :],
                    scale=1.0 / FEAT_SCALE,
                    accum_out=partials[0:B, g : g + 1],
                )

            nc.sync.dma_start(out[:, :], partials[:])
    nc.finalize()
    return nc


def _prep_host(inputs, features, indices, camids, camids_batch):
    """Host-side shard prep. Returns (per-core in_maps, fp64 target scores)."""
    f8 = ml_dtypes.float8_e4m3
    x = np.asarray(inputs, np.float32) / TEMP  # [B, D]
    cb = np.asarray(camids_batch).astype(np.int64)
    cn = np.asarray(camids).astype(np.int64)
    idx = np.asarray(indices).astype(np.int64)
    feats = np.asarray(features, np.float32)

    oh_b = (cb[:, None] == np.arange(NCAMS)[None, :]).astype(np.float32)  # [B, 8]

    # inp8[p, k, b] = x[b, k*128+p]; aug16 carries BIG*FEAT_SCALE one-hots.
    inp8 = np.ascontiguousarray(
        x.T.reshape(KC, 128, B).transpose(1, 0, 2).astype(f8)
    )
    aug16 = np.ascontiguousarray((BIG * FEAT_SCALE * oh_b).T.astype(np.float16))

    # exact target scores on host (fp64)
    tsel = np.einsum(
        "bd,bd->b", x.astype(np.float64), feats[idx].astype(np.float64)
    )

    in_maps = []
    for c in range(NCORES):
        sl = slice(c * N_SHARD, (c + 1) * N_SHARD)
        fT = (feats[sl].T * FEAT_SCALE).astype(f8)  # [D, 12500]
        fr = fT.reshape(KC, 128, N_SHARD)  # [k, p, j]
        fA = np.ascontiguousarray(
            fr[:, :, : NPAIR * 1000].reshape(KC, 128, NPAIR, 1000).transpose(2, 1, 0, 3)
        )
        fB = np.ascontiguousarray(fr[:, :, NPAIR * 1000 :].transpose(1, 0, 2))
        fC = np.ascontiguousarray(
            (cn[sl][None, :] == np.arange(NCAMS)[:, None]).astype(f8)
        )
        in_maps.append(
            {"featA": fA, "featB": fB, "featC": fC, "inp8": inp8, "aug16": aug16}
        )
    return in_maps, tsel


def _combine_host(results, tsel):
    """Cross-core logsumexp combine -> final scalar."""
    raw = np.stack([r["out"] for r in results]).astype(np.float64)  # [nc, B, 25]
    stot = raw.sum(axis=(0, 2))  # [B]
    lse = np.log(stot) + (C_SHIFT - BIG)  # = masked logsumexp of true scores
    nll = lse - tsel
    return np.float32(nll.mean())


_NC_CACHE = {}


def _get_nc(plan=PLAN):
    if plan not in _NC_CACHE:
        _NC_CACHE[plan] = build_nc(plan)
    return _NC_CACHE[plan]


def run_device(in_maps, plan=PLAN, **kwargs):
    nc = _get_nc(plan)
    return run_bass_kernel_spmd(
        nc, in_maps, core_ids=list(range(len(in_maps))), **kwargs
    )


def kernel(inputs, features, indices, camids, camids_batch):
    in_maps, tsel = _prep_host(inputs, features, indices, camids, camids_batch)
    res = run_device(in_maps)
    return _combine_host(res.results, tsel)


# revision 9
# speedup vs baseline: 1.2549x; 1.2013x over previous
"""Trainium2 Bass kernel for nn_Memory_22548578304755 (scatter_memory).

Computes: mean_b [ -log_softmax(mask(inputs @ features.T / temp))[b, indices[b]] ]

Strategy (8 NeuronCores, SPMD):
  - The host sorts the feature bank by camera id and deals each camera's
    rows round-robin across the 8 cores, padding each camera's per-core
    range to a common width ceil(N_c/8) with zero rows. Every core then
    holds the SAME column layout (camera c at columns [off_c, off_c+M_c)),
    so a single SPMD program serves all cores and the intra-camera mask
    disappears: exp-sums are accumulated per camera-pure column segment
    and the host picks each sample's own-camera denominator. Zero-pad
    columns contribute exp(-K_SHIFT) ~ 1e-44, i.e. nothing.
  - Features and the query block are quantized to fp8e4m3 (features
    scaled by 64), enabling DoubleRow matmuls: one instruction contracts
    2x128 rows, halving PE instruction count; per 500-col group only 8
    matmuls + 1-2 exp-activations remain.
  - The shard is laid out group-major ([NG, 128, KC, 500]) so every 1MB
    group DMA is a fully contiguous 8KB-per-partition read; group DMAs
    round-robin across DMA queues with a bounded lookahead.
  - Each exp-activation (ScalarE, scale=1/64 descale, bias=-K_SHIFT)
    accumulates its segment into a partials column; the host combines the
    8 cores' partials (cross-device logsumexp) and subtracts exact fp64
    target scores for the final scalar.
"""

import sys

import numpy as np

sys.path.insert(0, "/opt/trn_rl_repo")

import ml_dtypes  # noqa: E402

import concourse.bacc as bacc  # noqa: E402
import concourse.mybir as mybir  # noqa: E402
from concourse.tile import TileContext  # noqa: E402
from concourse.bass_utils import run_bass_kernel_spmd  # noqa: E402

B = 64
N = 100000
D = 2048
NCAMS = 8
TEMP = 0.07
NCORES = 8

K_SHIFT = 100.0  # shift so exp never overflows (max score ~64)
FEAT_SCALE = 64.0  # fp8 feature pre-scale (power of 2)

KC = D // 128  # 16 contraction chunks of 128 (8 DoubleRow pairs)
N_MM = 500  # group width (one PSUM bank)
LOOKAHEAD = 8  # group DMAs in flight ahead of compute
FEAT_BUFS = 10
N_WARM = 24  # PE warm-up matmuls (p-state ramp)

PLAN2 = ("sync", "gpsimd")
PLAN3 = ("sync", "gpsimd", "scalar")
PLAN = PLAN2


def _schedule(counts):
    """Uniform cross-core column layout + camera-pure activation pieces.

    counts: per-camera row counts over the full bank.
    Returns (widths, pieces) where widths[g] is group g's column count and
    pieces is a tuple of (group, a, b, cam) activation sub-ranges.
    """
    M_c = [(int(c) + NCORES - 1) // NCORES for c in counts]
    off = np.concatenate([[0], np.cumsum(M_c)])
    M_pad = int(off[-1])
    ngf = M_pad // N_MM
    extra = M_pad - ngf * N_MM
    widths = [N_MM] * ngf + ([extra] if extra else [])
    cuts = sorted(set([g * N_MM for g in range(len(widths))] + [M_pad] + [int(o) for o in off]))
    pieces = []
    for lo, hi in zip(cuts, cuts[1:]):
        g = lo // N_MM
        cam = int(np.searchsorted(off, lo, side="right") - 1)
        pieces.append((g, lo - g * N_MM, hi - g * N_MM, cam))
    return tuple(widths), tuple(pieces), off[:-1], M_pad


def build_nc(widths, pieces, plan=PLAN):
    """Build the single-core Bass program (identical across the 8 cores)."""
    dt = mybir.dt
    DR = mybir.MatmulPerfMode.DoubleRow
    nc = bacc.Bacc()

    ng = len(widths)
    ngf = sum(1 for w in widths if w == N_MM)
    extra = widths[-1] if ng > ngf else 0
    npieces = len(pieces)

    featG = nc.declare_dram_parameter("featG", [ngf, 128, KC, N_MM], dt.float8e4, False)
    if extra:
        featX = nc.declare_dram_parameter("featX", [128, KC, extra], dt.float8e4, False)
    inp8 = nc.declare_dram_parameter("inp8", [128, KC, B], dt.float8e4, False)
    out = nc.declare_dram_parameter("out", [B, npieces], dt.float32, True)

    by_group = [[] for _ in range(ng)]
    for i, (g, a, b, _cam) in enumerate(pieces):
        by_group[g].append((a, b, i))

    with TileContext(nc) as tc:
        with (
            tc.tile_pool(name="feat", bufs=FEAT_BUFS) as featp,
            tc.tile_pool(name="small", bufs=1) as smallp,
            tc.tile_pool(name="scratch", bufs=3) as scrp,
            tc.tile_pool(name="psum", bufs=6, space="PSUM") as psump,
            tc.tile_pool(name="warm", bufs=1, space="PSUM") as warmp,
        ):
            inp_t = smallp.tile([128, KC, B], dt.float8e4)
            nc.scalar.dma_start(inp_t[:], inp8[:, :, :])
            partials = smallp.tile([B, npieces], dt.float32)
            nbias = smallp.tile([B, 1], dt.float32)
            nc.vector.memset(nbias[:], -K_SHIFT)
            nc.vector.memset(partials[:], 0.0)

            queues = {"sync": nc.sync, "gpsimd": nc.gpsimd, "scalar": nc.scalar}
            fts = [None] * ng

            def issue(g):
                if g >= ng:
                    return
                w = widths[g]
                ft = featp.tile([128, KC, N_MM], dt.float8e4, tag="ft")
                src = featG[g, :, :, :] if w == N_MM else featX[:, :, :]
                queues[plan[g % len(plan)]].dma_start(ft[:, :, :w], src)
                fts[g] = ft

            for g in range(min(LOOKAHEAD, ng)):
                issue(g)

            # PE p-state warm-up: junk matmuls while the first group lands.
            wps = warmp.tile([B, B], dt.float32, tag="wps")
            for i in range(N_WARM):
                nc.tensor.matmul(
                    wps[:, :],
                    inp_t[:, 0:2, :],
                    inp_t[:, 0:2, :],
                    start=True,
                    stop=True,
                    perf_mode=DR,
                )

            for g in range(ng):
                w = widths[g]
                ft = fts[g]
                ps = psump.tile([B, N_MM], dt.float32, tag="ps")
                for kk in range(KC // 2):
                    nc.tensor.matmul(
                        ps[:, :w],
                        inp_t[:, 2 * kk : 2 * kk + 2, :],
                        ft[:, 2 * kk : 2 * kk + 2, :w],
                        start=(kk == 0),
                        stop=(kk == KC // 2 - 1),
                        perf_mode=DR,
                    )
                for a, b, i in by_group[g]:
                    ex = scrp.tile([B, N_MM], dt.float32, tag="ex")
                    nc.scalar.activation(
                        ex[:, : b - a],
                        ps[:, a:b],
                        mybir.ActivationFunctionType.Exp,
                        bias=nbias[:, :],
                        scale=1.0 / FEAT_SCALE,
                        accum_out=partials[:, i : i + 1],
                    )
                issue(g + LOOKAHEAD)

            nc.sync.dma_start(out[:, :], partials[:])
    nc.finalize()
    return nc


def _prep_host(inputs, features, indices, camids, camids_batch):
    """Host-side shard prep. Returns dict with in_maps, schedule, targets."""
    f8 = ml_dtypes.float8_e4m3
    x = np.asarray(inputs, np.float32) / TEMP  # [B, D]
    cb = np.asarray(camids_batch).astype(np.int64)
    cn = np.asarray(camids).astype(np.int64)
    idx = np.asarray(indices).astype(np.int64)
    feats = np.asarray(features, np.float32)

    counts = np.bincount(cn, minlength=NCAMS)
    widths, pieces, off, M_pad = _schedule(counts)
    ngf = sum(1 for w in widths if w == N_MM)
    extra = M_pad - ngf * N_MM

    # inp8[p, k, b] = x[b, k*128+p]
    inp8 = np.ascontiguousarray(x.T.reshape(KC, 128, B).transpose(1, 0, 2).astype(f8))

    # exact target scores on host (fp64)
    tsel = np.einsum("bd,bd->b", x.astype(np.float64), feats[idx].astype(np.float64))

    # quantized, transposed bank with a zero column at index N for padding
    F8 = np.empty((D, N + 1), f8)
    F8[:, :N] = (feats.T * FEAT_SCALE).astype(f8)
    F8[:, N] = 0

    # deal each camera's rows round-robin across cores at identical offsets
    order = np.argsort(cn, kind="stable")
    bounds = np.concatenate([[0], np.cumsum(counts)])
    colmap = np.full((NCORES, M_pad), N, np.int64)
    for c in range(NCAMS):
        rc = order[bounds[c] : bounds[c + 1]]
        j = np.arange(len(rc))
        colmap[j % NCORES, off[c] + j // NCORES] = rc

    in_maps = []
    for k in range(NCORES):
        fr = F8[:, colmap[k]].reshape(KC, 128, M_pad)
        fG = np.ascontiguousarray(
            fr[:, :, : ngf * N_MM].reshape(KC, 128, ngf, N_MM).transpose(2, 1, 0, 3)
        )
        m = {"featG": fG, "inp8": inp8}
        if extra:
            m["featX"] = np.ascontiguousarray(fr[:, :, ngf * N_MM :].transpose(1, 0, 2))
        in_maps.append(m)
    return {
        "in_maps": in_maps,
        "tsel": tsel,
        "cb": cb,
        "widths": widths,
        "pieces": pieces,
    }


def _combine_host(results, prep):
    """Cross-core logsumexp combine -> final scalar."""
    raw = np.stack([r["out"] for r in results]).astype(np.float64).sum(axis=0)  # [B, P]
    Dcam = np.zeros((B, NCAMS))
    for i, (_g, _a, _b, cam) in enumerate(prep["pieces"]):
        Dcam[:, cam] += raw[:, i]
    den = Dcam[np.arange(B), prep["cb"]]
    nll = np.log(den) + K_SHIFT - prep["tsel"]
    return np.float32(nll.mean())


_NC_CACHE = {}


def _get_nc(widths, pieces, plan=PLAN):
    key = (widths, pieces, plan)
    if key not in _NC_CACHE:
        _NC_CACHE[key] = build_nc(widths, pieces, plan)
    return _NC_CACHE[key]


def run_device(prep, plan=PLAN, **kwargs):
    nc = _get_nc(prep["widths"], prep["pieces"], plan)
    return run_bass_kernel_spmd(
        nc, prep["in_maps"], core_ids=list(range(len(prep["in_maps"]))), **kwargs
    )


def kernel(inputs, features, indices, camids, camids_batch):
    prep = _prep_host(inputs, features, indices, camids, camids_batch)
    res = run_device(prep)
    return _combine_host(res.results, prep)


# revision 23
# speedup vs baseline: 1.4494x; 1.1550x over previous
"""Trainium2 Bass kernel for nn_Memory_22548578304755 (scatter_memory).

Computes: mean_b [ -log_softmax(mask(inputs @ features.T / temp))[b, indices[b]] ]

Strategy (8 NeuronCores, SPMD):
  - The host sorts the feature bank by camera id and deals each camera's
    rows round-robin across the 8 cores, padding each camera's per-core
    range to a common width ceil(N_c/8) with zero rows. Every core then
    holds the SAME column layout (camera c at columns [off_c, off_c+M_c)),
    so a single SPMD program serves all cores and the intra-camera mask
    disappears: exp-sums are accumulated per camera-pure column segment
    and the host picks each sample's own-camera denominator. Zero-pad
    columns contribute exp(-K_SHIFT) ~ 1e-44, i.e. nothing.
  - Features and the query block are quantized to fp8e4m3 (features
    scaled by 64), enabling DoubleRow matmuls: one instruction contracts
    2x128 rows, halving PE instruction count; per 500-col group only 8
    matmuls + 1-2 exp-activations remain.
  - The shard is laid out group-major ([NG, 128, KC, 500]) so every 1MB
    group DMA is a fully contiguous 8KB-per-partition read; group DMAs
    round-robin across DMA queues with a bounded lookahead.
  - Each exp-activation (ScalarE, scale=1/64 descale, bias=-K_SHIFT)
    accumulates its segment into a partials column; the host combines the
    8 cores' partials (cross-device logsumexp) and subtracts exact fp64
    target scores for the final scalar.
"""

import sys

import numpy as np

sys.path.insert(0, "/opt/trn_rl_repo")

import ml_dtypes  # noqa: E402

import concourse.bacc as bacc  # noqa: E402
import concourse.mybir as mybir  # noqa: E402
from concourse.tile import TileContext  # noqa: E402
from concourse.bass_utils import run_bass_kernel_spmd  # noqa: E402

B = 64
N = 100000
D = 2048
NCAMS = 8
TEMP = 0.07
NCORES = 8

K_SHIFT = 100.0  # shift so exp never overflows (max score ~64)
FEAT_SCALE = 64.0  # fp8 feature pre-scale (power of 2)

KC = D // 128  # 16 contraction chunks of 128 (8 DoubleRow pairs)
N_MM = 500  # group width (one PSUM bank)
CG = "mixed"  # groups per DMA chunk: int, or "mixed" = [1,2,2,...,2,1]
LOOKAHEAD = 6  # chunk DMAs in flight ahead of compute
FEAT_BUFS = 7

PLAN = ("sync",)  # single HW DGE queue saturates the per-core HBM path


def _schedule(counts):
    """Uniform cross-core column layout + camera-pure activation pieces.

    counts: per-camera row counts over the full bank.
    Returns (widths, pieces) where widths[g] is group g's column count and
    pieces is a tuple of (group, a, b, cam) activation sub-ranges.
    """
    M_c = [(int(c) + NCORES - 1) // NCORES for c in counts]
    off = np.concatenate([[0], np.cumsum(M_c)])
    M_pad = int(off[-1])
    ngf = M_pad // N_MM
    extra = M_pad - ngf * N_MM
    if extra and extra <= 512 - N_MM:
        widths = [N_MM] * (ngf - 1) + [N_MM + extra]
    else:
        widths = [N_MM] * ngf + ([extra] if extra else [])
    if widths[0] >= 500:
        widths = [250, widths[0] - 250] + widths[1:]  # fast pipeline fill
    if widths[-1] >= 500:
        widths = widths[:-1] + [widths[-1] - 250, 250]  # small final quantum
    starts = np.concatenate([[0], np.cumsum(widths)])
    cuts = sorted(set([int(v) for v in starts] + [int(o) for o in off]))
    pieces = []
    for lo, hi in zip(cuts, cuts[1:]):
        g = int(np.searchsorted(starts, lo, side="right") - 1)
        cam = int(np.searchsorted(off, lo, side="right") - 1)
        pieces.append((g, lo - int(starts[g]), hi - int(starts[g]), cam))
    return tuple(widths), tuple(pieces), off[:-1], M_pad


def _chunks(widths, cg):
    """Partition group list into chunks. cg: int group count or "mixed"
    (1-group first/last chunks for fast pipeline fill/drain, 2 elsewhere)."""
    ng = len(widths)
    if cg == "mixed":
        sizes = [1, 1]
        while ng - sum(sizes) > 2:
            sizes.append(2)
        sizes += [1] * (ng - sum(sizes))
    else:
        sizes = []
        while sum(sizes) < ng:
            sizes.append(min(cg, ng - sum(sizes)))
    out = []
    g = 0
    for k in sizes:
        out.append((g, k, sum(widths[g + i] for i in range(k))))
        g += k
    return out


def build_nc(widths, pieces, plan=PLAN, cg=CG):
    """Build the single-core Bass program (identical across the 8 cores)."""
    dt = mybir.dt
    DR = mybir.MatmulPerfMode.DoubleRow
    nc = bacc.Bacc()

    ng = len(widths)
    npieces = len(pieces)
    chunks = _chunks(widths, cg)

    featCk = [
        nc.declare_dram_parameter(f"feat{ci}", [128, KC, w], dt.float8e4, False)
        for ci, (_g0, _k, w) in enumerate(chunks)
    ]
    inp8 = nc.declare_dram_parameter("inp8", [128, KC, B], dt.float8e4, False)
    init = nc.declare_dram_parameter("init", [B, 1 + npieces], dt.float32, False)
    out = nc.declare_dram_parameter("out", [B, npieces], dt.float32, True)

    by_group = [[] for _ in range(ng)]
    for i, (g, a, b, _cam) in enumerate(pieces):
        by_group[g].append((a, b, i))

    with TileContext(nc) as tc:
        with (
            tc.tile_pool(name="feat", bufs=FEAT_BUFS) as featp,
            tc.tile_pool(name="small", bufs=1) as smallp,
            tc.tile_pool(name="scratch", bufs=3) as scrp,
            tc.tile_pool(name="psum", bufs=7, space="PSUM") as psump,
        ):
            inp_t = smallp.tile([128, KC, B], dt.float8e4)
            init_t = smallp.tile([B, 1 + npieces], dt.float32)
            nc.scalar.dma_start(init_t[:], init[:, :])
            nbias = init_t[:, 0:1]
            partials = init_t[:, 1 : 1 + npieces]

            queues = {"sync": nc.sync, "gpsimd": nc.gpsimd, "scalar": nc.scalar}
            fts = [None] * len(chunks)
            wmax = max(w for _g0, _k, w in chunks)

            def issue(ci):
                if ci >= len(chunks):
                    return
                g0c, _k, w = chunks[ci]
                q = queues[plan[ci % len(plan)]]
                fta = featp.tile([128, KC // 2, wmax], dt.float8e4, tag="fta")
                ftb = featp.tile([128, KC // 2, wmax], dt.float8e4, tag="ftb")
                h = KC // 2
                if g0c % 2 == 0:  # first group ascends k: low half first
                    q.dma_start(fta[:, :, :w], featCk[ci][:, :h, :])
                    q.dma_start(ftb[:, :, :w], featCk[ci][:, h:, :])
                else:  # first group descends k: high half first
                    q.dma_start(ftb[:, :, :w], featCk[ci][:, h:, :])
                    q.dma_start(fta[:, :, :w], featCk[ci][:, :h, :])
                fts[ci] = (fta, ftb)

            issue(0)
            nc.sync.dma_start(inp_t[:], inp8[:, :, :])
            for ci in range(1, min(LOOKAHEAD, len(chunks))):
                issue(ci)

            g2c = {}
            for ci, (g0, k, _w) in enumerate(chunks):
                for i in range(k):
                    g2c[g0 + i] = (ci, i)
            for g in range(ng):
                w = widths[g]
                ci, gl = g2c[g]
                g0 = chunks[ci][0]
                fta, ftb = fts[ci]
                co = sum(widths[g0 + i] for i in range(gl))
                ps = psump.tile([B, 512], dt.float32, tag="ps")
                # k-snake: alternate contraction order so consecutive groups
                # share the boundary weight load.
                korder = range(KC // 2) if g % 2 == 0 else range(KC // 2 - 1, -1, -1)
                for j, kk in enumerate(korder):
                    half, kl = (fta, kk) if kk < KC // 4 else (ftb, kk - KC // 4)
                    nc.tensor.matmul(
                        ps[:, :w],
                        inp_t[:, 2 * kk : 2 * kk + 2, :],
                        half[:, 2 * kl : 2 * kl + 2, co : co + w],
                        start=(j == 0),
                        stop=(j == KC // 2 - 1),
                        perf_mode=DR,
                    )
                for a, b, i in by_group[g]:
                    ex = scrp.tile([B, 512], dt.float32, tag="ex")
                    nc.scalar.activation(
                        ex[:, : b - a],
                        ps[:, a:b],
                        mybir.ActivationFunctionType.Exp,
                        bias=nbias[:, :],
                        scale=1.0 / FEAT_SCALE,
                        accum_out=partials[:, i : i + 1],
                    )
                if gl == chunks[ci][1] - 1:
                    issue(ci + LOOKAHEAD)

            nc.scalar.dma_start(out[:, :], partials[:])
    nc.finalize()
    return nc


def _prep_host(inputs, features, indices, camids, camids_batch, cg=CG):
    """Host-side shard prep. Returns dict with in_maps, schedule, targets."""
    f8 = ml_dtypes.float8_e4m3
    x = np.asarray(inputs, np.float32) / TEMP  # [B, D]
    cb = np.asarray(camids_batch).astype(np.int64)
    cn = np.asarray(camids).astype(np.int64)
    idx = np.asarray(indices).astype(np.int64)
    feats = np.asarray(features, np.float32)

    counts = np.bincount(cn, minlength=NCAMS)
    widths, pieces, off, M_pad = _schedule(counts)
    ngf = sum(1 for w in widths if w == N_MM)
    extra = M_pad - ngf * N_MM

    # inp8[p, k, b] = x[b, k*128+p]
    inp8 = np.ascontiguousarray(x.T.reshape(KC, 128, B).transpose(1, 0, 2).astype(f8))

    # exact target scores on host (fp64)
    tsel = np.einsum("bd,bd->b", x.astype(np.float64), feats[idx].astype(np.float64))

    # quantized, transposed bank with a zero column at index N for padding
    F8 = np.empty((D, N + 1), f8)
    F8[:, :N] = (feats.T * FEAT_SCALE).astype(f8)
    F8[:, N] = 0

    # deal each camera's rows round-robin across cores at identical offsets
    order = np.argsort(cn, kind="stable")
    bounds = np.concatenate([[0], np.cumsum(counts)])
    colmap = np.full((NCORES, M_pad), N, np.int64)
    for c in range(NCAMS):
        rc = order[bounds[c] : bounds[c + 1]]
        j = np.arange(len(rc))
        colmap[j % NCORES, off[c] + j // NCORES] = rc

    chunks = _chunks(widths, cg)
    cb_cols = np.concatenate([[0], np.cumsum(widths)])
    init0 = np.zeros((B, 1 + len(pieces)), np.float32)
    init0[:, 0] = -K_SHIFT
    in_maps = []
    for k in range(NCORES):
        fr = F8[:, colmap[k]].reshape(KC, 128, M_pad)
        m = {"inp8": inp8, "init": init0}
        for ci, (g0, _kk, w) in enumerate(chunks):
            a = int(cb_cols[g0])
            m[f"feat{ci}"] = np.ascontiguousarray(fr[:, :, a : a + w].transpose(1, 0, 2))
        in_maps.append(m)
    return {
        "in_maps": in_maps,
        "tsel": tsel,
        "cb": cb,
        "widths": widths,
        "pieces": pieces,
        "cg": cg,
    }


def _combine_host(results, prep):
    """Cross-core logsumexp combine -> final scalar."""
    raw = np.stack([r["out"] for r in results]).astype(np.float64).sum(axis=0)  # [B, P]
    Dcam = np.zeros((B, NCAMS))
    for i, (_g, _a, _b, cam) in enumerate(prep["pieces"]):
        Dcam[:, cam] += raw[:, i]
    den = Dcam[np.arange(B), prep["cb"]]
    nll = np.log(den) + K_SHIFT - prep["tsel"]
    return np.float32(nll.mean())


_NC_CACHE = {}


def _get_nc(widths, pieces, plan=PLAN, cg=CG):
    key = (widths, pieces, plan, cg)
    if key not in _NC_CACHE:
        _NC_CACHE[key] = build_nc(widths, pieces, plan, cg)
    return _NC_CACHE[key]


def run_device(prep, plan=PLAN, **kwargs):
    nc = _get_nc(prep["widths"], prep["pieces"], plan, prep["cg"])
    return run_bass_kernel_spmd(
        nc, prep["in_maps"], core_ids=list(range(len(prep["in_maps"]))), **kwargs
    )


def kernel(inputs, features, indices, camids, camids_batch):
    prep = _prep_host(inputs, features, indices, camids, camids_batch)
    res = run_device(prep)
    return _combine_host(res.results, prep)
